# revision 1
# baseline (speedup 1.0000x reference)
"""Trainium2 Bass kernel for nn_GatingModule (noisy top-k MoE routing).

Strategy (data-parallel over 8 NeuronCores, 4096 tokens/core):
  - Host: transpose x -> [D, B_loc] and split into bf16 hi/lo planes;
    W_comb = [w_gate; w_noise]^T -> [D, 128] bf16 hi/lo.  3-pass bf16
    matmul (hi@hi + lo@hi + hi@lo) at 1 cycle/row gives ~fp32 accuracy.
  - Device: logits^T [e,t] via PE (W stationary, x^T moving, N=512),
    PE transpose back to [t,e], per-128-token-tile epilogue:
    softplus via ln(exp+1), noisy logits, top-8 via vector.max (one op),
    match_replace + reduce_max -> 9th value, softmax over top-8,
    normal-CDF argument v; erf batched at the end (single ACT table switch).
  - load: device returns per-(partition,expert) erf sums; host finishes
    0.5*B + 0.5*sum reduction. Gates assembled from per-core slices.
"""
import numpy as np
import ml_dtypes
from contextlib import ExitStack

import orjson
import concourse.bass as bass
import concourse.mybir as mybir
import concourse.tile as tile
from concourse.bass_utils import run_bass_kernel_spmd

F32 = mybir.dt.float32
BF16 = mybir.dt.bfloat16
AF = mybir.ActivationFunctionType
OP = mybir.AluOpType

B, D, E = 32768, 4096, 64
NCORES = 8
BLOC = B // NCORES          # 4096 tokens per core
NBLK = BLOC // 512          # 8 x 512-token blocks
NKC = D // 128              # 32 k-chunks
NTILE = BLOC // 128         # 32 x 128-token tiles
E2 = 2 * E                  # 128 = gate|noise combined output cols
NOISE_EPS = 0.01
INV_SQRT2 = 0.7071067811865476
NEG_BIG = -1e30

# ---------------------------------------------------------------- waitsplit
# This walrus build accepts ONE sync wait per instruction; Tile emits more.
# Hoist excess waits onto NoOps inserted right before the instruction.
_ws_counter = [0]


def _fix_bir_json_bytes(data: bytes) -> bytes:
    bir = orjson.loads(data)
    changed = False
    for fn in bir.get("functions", []):
        for bb in fn.get("blocks", []) or []:
            insts = bb.get("instructions")
            if not insts:
                continue
            out = []
            bchanged = False
            for ins in insts:
                si = ins.get("sync_info")
                waits = (si or {}).get("on_wait") or []
                if len(waits) > 1:
                    excess, keep = waits[:-1], waits[-1:]
                    si["on_wait"] = keep
                    for w in excess:
                        _ws_counter[0] += 1
                        out.append({
                            "name": f"I-ws-{_ws_counter[0]}",
                            "opcode": "NoOp",
                            "engine": ins.get("engine"),
                            "ins": [],
                            "outs": [],
                            "sync_info": {"on_wait": [w], "on_update": []},
                            "debug": ins.get("debug"),
                        })
                    bchanged = True
                out.append(ins)
            if bchanged:
                bb["instructions"] = out
                changed = True
    return orjson.dumps(bir) if changed else data


def _install_wait_fixup():
    if getattr(bass.Bass, "_wait_fixup_installed", False):
        return
    orig = bass.Bass.to_json_bytes

    def patched(self):
        return _fix_bir_json_bytes(orig(self))

    bass.Bass.to_json_bytes = patched
    bass.Bass._wait_fixup_installed = True


# ---------------------------------------------------------------- device IR
def _build():
    nc = bass.Bass()
    xh_d = nc.declare_dram_parameter("xT_hi", [D, BLOC], BF16, isOutput=False)
    xl_d = nc.declare_dram_parameter("xT_lo", [D, BLOC], BF16, isOutput=False)
    wh_d = nc.declare_dram_parameter("wT_hi", [D, E2], BF16, isOutput=False)
    wl_d = nc.declare_dram_parameter("wT_lo", [D, E2], BF16, isOutput=False)
    bc_d = nc.declare_dram_parameter("b_comb", [E2, 1], F32, isOutput=False)
    nz_d = nc.declare_dram_parameter("noise", [BLOC, E], F32, isOutput=False)
    id_d = nc.declare_dram_parameter("ident", [128, 128], F32, isOutput=False)
    g_d = nc.declare_dram_parameter("gates", [BLOC, E], F32, isOutput=True)
    a_d = nc.declare_dram_parameter("accerf", [128, E], F32, isOutput=True)

    with ExitStack() as ctx:
        tc = ctx.enter_context(tile.TileContext(nc))
        const_pool = ctx.enter_context(tc.tile_pool(name="consts", bufs=1))
        xpool = ctx.enter_context(tc.tile_pool(name="xslab", bufs=2))
        lpool = ctx.enter_context(tc.tile_pool(name="logits", bufs=2))
        epool = ctx.enter_context(tc.tile_pool(name="epi", bufs=3))
        vpool = ctx.enter_context(tc.tile_pool(name="vbuf", bufs=1))
        mm_ps = ctx.enter_context(tc.tile_pool(name="mmps", bufs=2, space="PSUM"))
        tr_ps = ctx.enter_context(tc.tile_pool(name="trps", bufs=4, space="PSUM"))

        # constants / weights
        w_hi = const_pool.tile([128, NKC, E2], BF16)
        nc.sync.dma_start(w_hi[:], wh_d[:].rearrange("(kc p) e -> p kc e", p=128))
        w_lo = const_pool.tile([128, NKC, E2], BF16)
        nc.sync.dma_start(w_lo[:], wl_d[:].rearrange("(kc p) e -> p kc e", p=128))
        bcomb = const_pool.tile([128, 1], F32)
        nc.sync.dma_start(bcomb[:], bc_d[:])
        ident = const_pool.tile([128, 128], F32)
        nc.sync.dma_start(ident[:], id_d[:])
        eps_t = const_pool.tile([128, 1], F32)
        nc.vector.memset(eps_t[:], NOISE_EPS)

        # v-buffer for the deferred erf pass: [128, NTILE, 64]
        vbuf = vpool.tile([128, NTILE, E], F32)

        for blk in range(NBLK):
            # x^T slabs for this 512-token block (hi + lo planes)
            xh = xpool.tile([128, NKC, 512], BF16, tag="xh")
            nc.sync.dma_start(
                xh[:],
                xh_d[:, blk * 512:(blk + 1) * 512].rearrange(
                    "(kc p) t -> p kc t", p=128),
            )
            xl = xpool.tile([128, NKC, 512], BF16, tag="xl")
            nc.sync.dma_start(
                xl[:],
                xl_d[:, blk * 512:(blk + 1) * 512].rearrange(
                    "(kc p) t -> p kc t", p=128),
            )
            # 3-pass accumulating matmul -> logits^T [e2, 512] in PSUM
            pmm = mm_ps.tile([128, 512], F32)
            n_mm = 3 * NKC
            i_mm = 0
            for kc in range(NKC):
                for wt, xt in ((w_hi, xh), (w_lo, xh), (w_hi, xl)):
                    nc.tensor.matmul(
                        pmm[:], wt[:, kc, :], xt[:, kc, :],
                        start=(i_mm == 0), stop=(i_mm == n_mm - 1),
                    )
                    i_mm += 1
            # PSUM -> SBUF with per-expert bias add
            lt = lpool.tile([128, 512], F32, tag="lt")
            nc.scalar.activation(lt[:], pmm[:], AF.Identity, bias=bcomb[:, 0:1])

            for q in range(4):
                t_idx = blk * 4 + q
                # transpose [e2, 128] -> [t, e2]
                ptr = tr_ps.tile([128, 128], F32, tag="ptr")
                nc.tensor.transpose(ptr[:], lt[:, q * 128:(q + 1) * 128], ident[:])
                te = epool.tile([128, 128], F32, tag="te")
                nc.scalar.copy(te[:], ptr[:])
                clean = te[:, :E]
                pre = te[:, E:]

                nz = epool.tile([128, E], F32, tag="nz")
                nc.sync.dma_start(nz[:], nz_d[t_idx * 128:(t_idx + 1) * 128, :])

                # softplus = ln(exp(pre)+1); inv_std = exp(-ln(sp+eps))
                esp = epool.tile([128, E], F32, tag="esp")
                nc.scalar.activation(esp[:], pre, AF.Exp)
                sp = epool.tile([128, E], F32, tag="sp")
                nc.scalar.activation(sp[:], esp[:], AF.Ln, bias=1.0)
                lnstd = epool.tile([128, E], F32, tag="lnstd")
                nc.scalar.activation(lnstd[:], sp[:], AF.Ln, bias=eps_t[:, 0:1])
                inv_std = epool.tile([128, E], F32, tag="invstd")
                nc.scalar.activation(inv_std[:], lnstd[:], AF.Exp, scale=-1.0)
                # noisy = (sp+eps)*noise + clean
                nstd = epool.tile([128, E], F32, tag="nstd")
                nc.vector.scalar_tensor_tensor(nstd[:], sp[:], NOISE_EPS, nz[:],
                                               op0=OP.add, op1=OP.mult)
                noisy = epool.tile([128, E], F32, tag="noisy")
                nc.vector.tensor_tensor(noisy[:], nstd[:], clean, op=OP.add)
                # top-8 (desc) + 9th
                max8 = epool.tile([128, 8], F32, tag="max8")
                nc.vector.max(max8[:], noisy[:])
                zap = epool.tile([128, E], F32, tag="zap")
                nc.vector.match_replace(zap[:], in_to_replace=max8[:],
                                        in_values=noisy[:], imm_value=NEG_BIG)
                t9 = epool.tile([128, 1], F32, tag="t9")
                nc.vector.reduce_max(t9[:], zap[:], axis=mybir.AxisListType.X)
                # softmax over top-8 folded into one exp
                neg_m1 = epool.tile([128, 1], F32, tag="negm1")
                nc.vector.tensor_scalar(neg_m1[:], max8[:, 0:1], -1.0, None,
                                        op0=OP.mult)
                e8 = epool.tile([128, 8], F32, tag="e8")
                denom = epool.tile([128, 1], F32, tag="denom")
                nc.scalar.activation(e8[:], max8[:], AF.Exp, bias=neg_m1[:, 0:1],
                                     accum_out=denom[:])
                lnden = epool.tile([128, 1], F32, tag="lnden")
                nc.scalar.activation(lnden[:], denom[:], AF.Ln)
                bias2 = epool.tile([128, 1], F32, tag="bias2")
                nc.vector.tensor_tensor(bias2[:], neg_m1[:], lnden[:],
                                        op=OP.subtract)
                e2t = epool.tile([128, E], F32, tag="e2t")
                nc.scalar.activation(e2t[:], noisy[:], AF.Exp, bias=bias2[:, 0:1])
                mask = epool.tile([128, E], F32, tag="mask")
                nc.vector.tensor_scalar(mask[:], noisy[:], max8[:, 7:8], None,
                                        op0=OP.is_ge)
                gates = epool.tile([128, E], F32, tag="gates")
                nc.vector.tensor_tensor(gates[:], e2t[:], mask[:], op=OP.mult)
                nc.sync.dma_start(g_d[t_idx * 128:(t_idx + 1) * 128, :], gates[:])
                # v = ((clean + mask*(t8-t9)) - t8) * inv_std  (erf deferred)
                dpos = epool.tile([128, 1], F32, tag="dpos")
                nc.vector.tensor_tensor(dpos[:], max8[:, 7:8], t9[:],
                                        op=OP.subtract)
                u = epool.tile([128, E], F32, tag="u")
                nc.vector.scalar_tensor_tensor(u[:], mask[:], dpos[:, 0:1], clean,
                                               op0=OP.mult, op1=OP.add)
                nc.vector.scalar_tensor_tensor(
                    vbuf[:, t_idx, :], u[:], max8[:, 7:8], inv_std[:],
                    op0=OP.subtract, op1=OP.mult)

        # deferred erf pass: one table switch, one big op, tree-reduce
        erfv = vpool.tile([128, NTILE, E], F32)
        nc.scalar.activation(erfv[:], vbuf[:], AF.Erf, scale=INV_SQRT2)
        n = NTILE
        while n > 1:
            h = n // 2
            nc.vector.tensor_tensor(
                erfv[:, :h, :], erfv[:, :h, :], erfv[:, h:n, :], op=OP.add)
            n = h
        nc.sync.dma_start(a_d[:], erfv[:, 0, :])

    return nc


_NC_CACHE = [None]


def kernel(x, w_gate, b_gate, w_noise, b_noise, noise):
    _install_wait_fixup()
    x = np.asarray(x, np.float32)
    noise_f = np.asarray(noise, np.float32)
    wc = np.concatenate([np.asarray(w_gate, np.float32),
                         np.asarray(w_noise, np.float32)], axis=0)  # [128, D]
    wT = np.ascontiguousarray(wc.T)                                  # [D, 128]
    wT_hi = wT.astype(ml_dtypes.bfloat16)
    wT_lo = (wT - wT_hi.astype(np.float32)).astype(ml_dtypes.bfloat16)
    b_comb = np.concatenate([np.asarray(b_gate, np.float32),
                             np.asarray(b_noise, np.float32)])[:, None]
    ident = np.eye(128, dtype=np.float32)

    in_maps = []
    for c in range(NCORES):
        xc = x[c * BLOC:(c + 1) * BLOC]                 # [BLOC, D]
        xT = np.ascontiguousarray(xc.T)                 # [D, BLOC]
        xT_hi = xT.astype(ml_dtypes.bfloat16)
        xT_lo = (xT - xT_hi.astype(np.float32)).astype(ml_dtypes.bfloat16)
        in_maps.append({
            "xT_hi": xT_hi, "xT_lo": xT_lo,
            "wT_hi": wT_hi, "wT_lo": wT_lo,
            "b_comb": b_comb, "ident": ident,
            "noise": np.ascontiguousarray(noise_f[c * BLOC:(c + 1) * BLOC]),
        })

    if _NC_CACHE[0] is None:
        _NC_CACHE[0] = _build()
    res = run_bass_kernel_spmd(_NC_CACHE[0], in_maps,
                               core_ids=list(range(NCORES))).results

    gates = np.concatenate([res[c]["gates"] for c in range(NCORES)], axis=0)
    erf_sum = np.zeros(E, dtype=np.float64)
    for c in range(NCORES):
        erf_sum += res[c]["accerf"].astype(np.float64).sum(axis=0)
    load = (0.5 * B + 0.5 * erf_sum).astype(np.float32)
    return gates.astype(np.float32), load
